# revision 12
# baseline (speedup 1.0000x reference)
"""Trainium2 Bass kernel for DeterministicLSTMSensorBasedForwardDynamics.

Problem: B=4096, T=50, OBS=64, ACT=16, H=256, OUT=64, 5-layer MLP head.
  x = concat(traj, act)                     [B, T, 80]
  LSTM over T with silu cell activation (g = silu(zg), h = o * silu(c))
  MLP: 5 x (Dense(256) + silu), Dense(64)

Strategy (data parallel over 8 cores, 512 batch each):
  * Everything runs in "transposed" layout: activations are [feature, batch]
    so LSTM/MLP weights are the PE-stationary operand and the batch streams.
  * z^T per step lives in PSUM as [128, 2048] = 4 banks = gates (i, f, o, g)
    (weight columns permuted on host so sigmoid-gates i,f,o are contiguous).
  * sigmoid(x) = 0.5*(1 + tanh(x/2)): tanh and silu share one ACT table set,
    so no table reloads.  The 0.5 factors are folded away by keeping
    c' = 2c, h' = 2h as the on-chip state with Wh, mlp_W[0] pre-halved on
    host.  Each gate update is then a single fused scalar_tensor_tensor.
  * The batch is split in two 256-wide chunks that pipeline PE (matmuls)
    against ACT (tanh/silu) and DVE (gate arithmetic).
  * bh is folded into the Wi matmul via an all-ones input row (K=81).
  * Host prep builds x^T as bf16 [128, 4*50*128]: partitions = padded input
    feature, free = (btile, t, b).  Matmuls run in bf16; MLP in fp32r.
"""

import sys

sys.path.insert(0, "/opt/trn_rl_repo")

import numpy as np
import ml_dtypes

import concourse.bacc as bacc
import concourse.tile as tile
from concourse import mybir
from concourse import bass_utils

N_CORES = 8
B, T, OBS, ACTD, H, OUT, NL = 4096, 50, 64, 16, 256, 64, 5
BC = B // N_CORES          # batch per core = 512
NBT = BC // 128            # b-tiles per core = 4
CHUNK = 256                # batch per pipeline chunk
NCH = BC // CHUNK          # chunks = 2

BF16 = mybir.dt.bfloat16
F32 = mybir.dt.float32
F32R = mybir.dt.float32r
AF = mybir.ActivationFunctionType
ALU = mybir.AluOpType

# gate permutation: reference order (i, f, g, o) -> bank order (g, i, f, o)
_PERM = np.concatenate([np.arange(512, 768), np.arange(0, 512),
                        np.arange(768, 1024)])

_CACHE = {}


def _build(t_steps=T):
    """Build + compile the Bass module (cached)."""
    if t_steps in _CACHE:
        return _CACHE[t_steps]

    nc = bacc.Bacc("TRN2", target_bir_lowering=False, debug=False,
                   num_devices=N_CORES)

    xt_d = nc.dram_tensor("xt", [128, NBT * t_steps * 128], BF16,
                          kind="ExternalInput").ap()
    wh_d = nc.dram_tensor("wh", [128, 2048], BF16, kind="ExternalInput").ap()
    wi_d = nc.dram_tensor("wi", [128, 1024], BF16, kind="ExternalInput").ap()
    mlpw_d = nc.dram_tensor("mlpw", [128, NL * 2 * 256], F32R,
                            kind="ExternalInput").ap()
    mlpb_d = nc.dram_tensor("mlpb", [128, NL * 2], F32,
                            kind="ExternalInput").ap()
    wout_d = nc.dram_tensor("wout", [128, 128], F32R, kind="ExternalInput").ap()
    boutb_d = nc.dram_tensor("boutb", [128, 256], F32,
                             kind="ExternalInput").ap()
    pred_d = nc.dram_tensor("pred", [BC, OUT], F32, kind="ExternalOutput").ap()

    with tile.TileContext(nc) as tc:
        with (
            tc.tile_pool(name="singles", bufs=1) as singles,
            tc.tile_pool(name="hpool", bufs=3) as hpool,
            tc.tile_pool(name="work", bufs=3) as work,
            tc.tile_pool(name="psum", bufs=1, space="PSUM") as psum,
        ):
            # ---- weights / persistent state ----
            wh = singles.tile([128, 2048], BF16, tag="wh")
            nc.sync.dma_start(wh[:], wh_d[:])
            wi = singles.tile([128, 1024], BF16, tag="wi")
            nc.sync.dma_start(wi[:], wi_d[:])
            mlpw = singles.tile([128, NL * 2 * 256], F32R, tag="mlpw")
            nc.sync.dma_start(mlpw[:], mlpw_d[:])
            mlpb = singles.tile([128, NL * 2], F32, tag="mlpb")
            nc.sync.dma_start(mlpb[:], mlpb_d[:])
            wout = singles.tile([128, 128], F32R, tag="wout")
            nc.sync.dma_start(wout[:], wout_d[:])
            boutb = singles.tile([128, 256], F32, tag="boutb")
            nc.sync.dma_start(boutb[:], boutb_d[:])

            # x^T, loaded in chunks so step 0 can start early
            xt = singles.tile([128, NBT * t_steps * 128], BF16, tag="xt")
            TBLK = 10 if t_steps % 10 == 0 else t_steps
            for bt in range(NBT):
                for t0 in range(0, t_steps, TBLK):
                    lo = bt * t_steps * 128 + t0 * 128
                    hi = lo + TBLK * 128
                    nc.sync.dma_start(xt[:, lo:hi], xt_d[:, lo:hi])
            xt_r = xt[:].rearrange("p (bt t b) -> p bt t b", bt=NBT, t=t_steps)

            # c' = 2c state per chunk (written at t=0, no memset needed)
            cst = [singles.tile([128, CHUNK * 2], F32, tag=f"c{c}",
                                name=f"c{c}")
                   for c in range(NCH)]
            # h'_final for the MLP: [128, (ktile 2) * 512]
            hlast = singles.tile([128, 1024], F32R, tag="hlast")

            def lstm_step(t, ch, h_prev):
                # psum banks: 0 = g-gate, 1 = i, 2 = f, 3 = o
                c_t = cst[ch]
                zp = psum.tile([128, 2048], F32, tag=f"z{ch}")
                rhs_x = xt_r[0:81, 2 * ch:2 * ch + 2, t:t + 1, :]
                # x-matmuls first: independent of h, can run during the
                # previous step's gate math (only needs the psum slot free)
                # One accumulation group per PSUM bank (start=True zeroes the
                # whole 2KB bank lazily).  The x-matmuls open each bank's
                # group first — they don't depend on h, so they run during
                # the previous step's gate math; the h-matmuls close the
                # groups in bank order (g, i, f) with o last.
                for m in range(8):
                    nc.tensor.matmul(
                        zp[:, m * 256:(m + 1) * 256],
                        wi[0:81, m * 128:(m + 1) * 128], rhs_x,
                        start=(m % 2 == 0), stop=(t == 0 and m % 2 == 1))
                if t > 0:
                    for m in range(8):
                        o_ap = zp[:, m * 256:(m + 1) * 256]
                        nc.tensor.matmul(
                            o_ap, wh[:, m * 128:(m + 1) * 128],
                            h_prev[:, 0:256], start=False, stop=False)
                        nc.tensor.matmul(
                            o_ap, wh[:, 1024 + m * 128:1024 + (m + 1) * 128],
                            h_prev[:, 256:512], start=False,
                            stop=(m % 2 == 1))

                g = work.tile([128, 512], BF16, tag=f"g{ch}")
                nc.scalar.activation(g[:], zp[:, 0:512], AF.Silu)
                tau = work.tile([128, 1024], BF16, tag=f"tau{ch}")
                nc.scalar.activation(tau[:], zp[:, 512:1536], AF.Tanh,
                                     scale=0.5)
                tau_o = work.tile([128, 512], BF16, tag=f"tau_o{ch}")
                nc.scalar.activation(tau_o[:], zp[:, 1536:2048], AF.Tanh,
                                     scale=0.5)

                if t > 0:
                    u = work.tile([128, 512], F32, tag=f"u{ch}")
                    nc.vector.scalar_tensor_tensor(
                        u[:], tau[:, 512:1024], 1.0, c_t[:], ALU.add, ALU.mult)
                    tt = work.tile([128, 512], BF16, tag=f"t{ch}")
                    nc.vector.scalar_tensor_tensor(
                        tt[:], tau[:, 0:512], 1.0, g[:], ALU.add, ALU.mult)
                    nc.vector.scalar_tensor_tensor(
                        c_t[:], u[:], 0.5, tt[:], ALU.mult, ALU.add)
                else:
                    nc.vector.scalar_tensor_tensor(
                        c_t[:], tau[:, 0:512], 1.0, g[:], ALU.add, ALU.mult)

                # silu(c) and h' split by h-row-tile so the next step's
                # k0 matmuls can start as soon as h rows 0:128 are ready
                sc = work.tile([128, 512], BF16, tag=f"sc{ch}")
                last = t == t_steps - 1
                if not last:
                    h_new = hpool.tile([128, 512], BF16, tag=f"h{ch}")
                for r in range(2):
                    sl = slice(r * 256, (r + 1) * 256)
                    nc.scalar.activation(sc[:, sl], c_t[:, sl], AF.Silu,
                                         scale=0.5)
                    if not last:
                        nc.vector.scalar_tensor_tensor(
                            h_new[:, sl], tau_o[:, sl], 1.0, sc[:, sl],
                            ALU.add, ALU.mult)
                    else:
                        o_ap = hlast[:, r * 512 + ch * CHUNK:
                                     r * 512 + (ch + 1) * CHUNK]
                        nc.vector.scalar_tensor_tensor(
                            o_ap, tau_o[:, sl], 1.0, sc[:, sl],
                            ALU.add, ALU.mult)
                return None if last else h_new

            hprev = [None] * NCH
            for t in range(t_steps):
                for ch in range(NCH):
                    hprev[ch] = lstm_step(t, ch, hprev[ch])

            # ---- MLP head (fp32r, full 512 batch) ----
            cur = hlast
            for layer in range(NL):
                mp = psum.tile([128, 1024], F32, tag=f"z{layer % 2}",
                               name=f"mlp_ps{layer}")
                for m in range(2):
                    for k in range(2):
                        nc.tensor.matmul(
                            mp[:, m * 512:(m + 1) * 512],
                            mlpw[:, (layer * 2 + k) * 256 + m * 128:
                                 (layer * 2 + k) * 256 + (m + 1) * 128
                                 ],
                            cur[:, k * 512:(k + 1) * 512],
                            start=(k == 0), stop=(k == 1))
                nxt = work.tile([128, 1024], F32R, tag="mlp_out")
                for m in range(2):
                    nc.scalar.activation(
                        nxt[:, m * 512:(m + 1) * 512],
                        mp[:, m * 512:(m + 1) * 512], AF.Silu,
                        bias=mlpb[:, layer * 2 + m:layer * 2 + m + 1])
                cur = nxt

            # output layer back to [batch, OUT] layout:
            # lhsT = activations (stationary), rhs = Wout (moving)
            # all 8 matmuls share one PSUM bank -> one accumulation group
            pp = psum.tile([128, 256], F32, tag=f"z{NL % 2}", name="pred_ps")
            for m in range(4):
                for k in range(2):
                    nc.tensor.matmul(
                        pp[:, m * 64:(m + 1) * 64],
                        cur[:, k * 512 + m * 128:k * 512 + (m + 1) * 128
                            ],
                        wout[:, k * 64:(k + 1) * 64],
                        start=(m == 0 and k == 0), stop=(m == 3 and k == 1))
            preds = singles.tile([128, 256], F32, tag="preds")
            nc.vector.tensor_add(preds[:], pp[:], boutb[:])
            nc.sync.dma_start(
                pred_d.rearrange("(m p) f -> p m f", p=128),
                preds[:].rearrange("p (m f) -> p m f", f=OUT))

    nc.compile()
    _CACHE[t_steps] = nc
    return nc


def _prep_inputs(trajectory, actions, Wi, Wh, bh, mlp_W, mlp_b, Wout, bout,
                 t_steps=T):
    """Host-side layout prep. Returns per-core input maps."""
    f32 = np.float32
    trajectory = np.asarray(trajectory, f32)
    actions = np.asarray(actions, f32)
    Wi = np.asarray(Wi, f32)
    Wh = np.asarray(Wh, f32)
    bh = np.asarray(bh, f32)
    mlp_W = np.asarray(mlp_W, f32)
    mlp_b = np.asarray(mlp_b, f32)
    Wout = np.asarray(Wout, f32)
    bout = np.asarray(bout, f32)

    # gate permutation + h'=2h folding
    Wh_p = (0.5 * Wh[:, _PERM]).astype(ml_dtypes.bfloat16)
    Wi_p = Wi[:, _PERM]
    bh_p = bh[_PERM]

    wh_l = Wh_p.reshape(2, 128, 1024).transpose(1, 0, 2).reshape(128, 2048)
    wi_l = np.zeros((128, 1024), ml_dtypes.bfloat16)
    wi_l[0:OBS] = Wi_p[0:OBS].astype(ml_dtypes.bfloat16)
    wi_l[OBS:OBS + ACTD] = Wi_p[OBS:OBS + ACTD].astype(ml_dtypes.bfloat16)
    wi_l[80] = bh_p.astype(ml_dtypes.bfloat16)

    mw = mlp_W.copy()
    mw[0] = mw[0] * 0.5  # h' = 2h fold
    mlpw_l = mw.reshape(NL, 2, 128, 256).transpose(2, 0, 1, 3).reshape(
        128, NL * 2 * 256)
    mlpb_l = mlp_b.reshape(NL, 2, 128).transpose(2, 0, 1).reshape(128, NL * 2)
    wout_l = Wout.reshape(2, 128, 64).transpose(1, 0, 2).reshape(128, 128)
    boutb_l = np.tile(bout, (128, 4))

    in_maps = []
    for c in range(N_CORES):
        tr = trajectory[c * BC:(c + 1) * BC, :t_steps]    # [512, t, 64]
        ac = actions[c * BC:(c + 1) * BC, :t_steps]       # [512, t, 16]
        xt = np.zeros((128, NBT, t_steps, 128), ml_dtypes.bfloat16)
        xt[0:OBS] = tr.reshape(NBT, 128, t_steps, OBS).transpose(
            3, 0, 2, 1).astype(ml_dtypes.bfloat16)
        xt[OBS:OBS + ACTD] = ac.reshape(NBT, 128, t_steps, ACTD).transpose(
            3, 0, 2, 1).astype(ml_dtypes.bfloat16)
        xt[80] = 1.0
        in_maps.append({
            "xt": xt.reshape(128, NBT * t_steps * 128),
            "wh": wh_l, "wi": wi_l, "mlpw": mlpw_l.astype(f32),
            "mlpb": mlpb_l.astype(f32), "wout": wout_l.astype(f32),
            "boutb": boutb_l.astype(f32),
        })
    return in_maps


_RUNNER = {}


def _get_runner(t_steps=T):
    """Build the bass module once and wrap it in a cached, reusable
    shard-mapped PJRT executable (one NEFF compile per process)."""
    if t_steps in _RUNNER:
        return _RUNNER[t_steps]

    import jax
    from jax.sharding import Mesh, PartitionSpec
    from jax.experimental.shard_map import shard_map
    from concourse import bass2jax, mybir as _mb

    nc = _build(t_steps)
    bass2jax.install_neuronx_cc_hook()

    part_name = (nc.partition_id_tensor.name if nc.partition_id_tensor
                 else None)
    in_names, out_names, out_avals = [], [], []
    for alloc in nc.m.functions[0].allocations:
        if not isinstance(alloc, _mb.MemoryLocationSet):
            continue
        name = alloc.memorylocations[0].name
        if alloc.kind == "ExternalInput":
            if name != part_name:
                in_names.append(name)
        elif alloc.kind == "ExternalOutput":
            out_names.append(name)
            out_avals.append(jax.core.ShapedArray(
                tuple(alloc.tensor_shape), _mb.dt.np(alloc.dtype)))
    n_params = len(in_names)
    n_outs = len(out_avals)
    all_names = in_names + out_names
    if part_name is not None:
        all_names = all_names + [part_name]

    def _body(*args):
        operands = list(args)
        if part_name is not None:
            operands.append(bass2jax.partition_id_tensor())
        outs = bass2jax._bass_exec_p.bind(
            *operands,
            out_avals=tuple(out_avals),
            in_names=tuple(all_names),
            out_names=tuple(out_names),
            lowering_input_output_aliases=(),
            sim_require_finite=True,
            sim_require_nnan=True,
            nc=nc,
        )
        return tuple(outs)

    devices = jax.devices()[:N_CORES]
    mesh = Mesh(np.asarray(devices), ("core",))
    donate = tuple(range(n_params, n_params + n_outs))
    sharded = jax.jit(
        shard_map(_body, mesh=mesh,
                  in_specs=(PartitionSpec("core"),) * (n_params + n_outs),
                  out_specs=(PartitionSpec("core"),) * n_outs,
                  check_rep=False),
        donate_argnums=donate, keep_unused=True)

    out_shapes = [(a.shape, a.dtype) for a in out_avals]

    def run(in_maps):
        concat_in = [
            np.concatenate([np.asarray(in_maps[c][nm]) for c in
                            range(N_CORES)], axis=0)
            for nm in in_names
        ]
        zeros = [np.zeros((N_CORES * s[0],) + tuple(s[1:]), dt)
                 for s, dt in out_shapes]
        outs = sharded(*concat_in, *zeros)
        return {nm: np.asarray(outs[i]) for i, nm in enumerate(out_names)}

    _RUNNER[t_steps] = run
    return run


def kernel(trajectory, actions, Wi, Wh, bh, mlp_W, mlp_b, Wout, bout):
    run = _get_runner(T)
    in_maps = _prep_inputs(trajectory, actions, Wi, Wh, bh, mlp_W, mlp_b,
                           Wout, bout, T)
    pred = run(in_maps)["pred"]          # [8*512, 64] already batch-ordered
    return pred.astype(np.float32)


# revision 32
# speedup vs baseline: 65122.8661x; 65122.8661x over previous
"""Trainium2 Bass kernel for DeterministicLSTMSensorBasedForwardDynamics.

Problem: B=4096, T=50, OBS=64, ACT=16, H=256, OUT=64, 5-layer MLP head.
  x = concat(traj, act)                     [B, T, 80]
  LSTM over T with silu cell activation (g = silu(zg), h = o * silu(c))
  MLP: 5 x (Dense(256) + silu), Dense(64)

Strategy (data parallel over 8 cores, 512 batch each):
  * Everything runs in "transposed" layout: activations are [feature, batch]
    so LSTM/MLP weights are the PE-stationary operand and the 512-batch
    streams as the matmul moving dimension.
  * z^T per step fills all 8 PSUM banks; bank m holds z-rows m*128:(m+1)*128
    = (gate, h-row-tile) pairs in order (g0 g1 f0 f1 i0 i1 o0 o1) via a host
    weight-column permutation.  One accumulation group per bank, opened by
    the x-matmuls (h-independent, so they run during the previous step's
    gate phase) and closed by the k1 h-matmuls.
  * sigmoid(x) = 0.5*(1 + tanh(x/2)): tanh and silu share one ACT table set,
    so no table reloads.  The 0.5 factors are folded away by keeping
    c' = 2c, h' = 2h as the on-chip state with Wh, mlp_W[0] pre-halved on
    host.  Each gate update is then a single fused scalar_tensor_tensor.
  * The gate tail is split by h-row-tile into two sub-chains so the next
    step's k0 matmuls start as soon as h rows 0:128 are ready; matmul and
    ACT-read orders are matched to the per-bank PSUM release ladder.
  * bh is folded into the Wi matmul via an all-ones input row (K=81).
  * Host prep builds x^T as bf16 [128, 4*50*128]: partitions = padded input
    feature, free = (btile, t, b).  Matmuls run in bf16; MLP in fp32r.
  * ACT reads of PSUM must be bank-contiguous: strided cross-bank ACT access
    patterns defeat the bank-overlap tracking and crash the device.
"""

import sys

sys.path.insert(0, "/opt/trn_rl_repo")

import numpy as np
import ml_dtypes

import concourse.bacc as bacc
import concourse.tile as tile
from concourse import mybir
from concourse import bass_utils

N_CORES = 8
B, T, OBS, ACTD, H, OUT, NL = 4096, 50, 64, 16, 256, 64, 5
BC = B // N_CORES          # batch per core = 512
NBT = BC // 128            # b-tiles per core = 4
CHUNK = 512                # single chunk: batch streams as N=512
NCH = BC // CHUNK          # = 1

BF16 = mybir.dt.bfloat16
F32 = mybir.dt.float32
F32R = mybir.dt.float32r
AF = mybir.ActivationFunctionType
ALU = mybir.AluOpType

# gate permutation: reference order (i, f, g, o) -> bank order (g, f, i, o)
_PERM = np.concatenate([np.arange(512, 768), np.arange(256, 512),
                        np.arange(0, 256), np.arange(768, 1024)])

_CACHE = {}


def _build(t_steps=T):
    """Build + compile the Bass module (cached)."""
    if t_steps in _CACHE:
        return _CACHE[t_steps]

    nc = bacc.Bacc("TRN2", target_bir_lowering=False, debug=False,
                   num_devices=N_CORES)

    xt_d = nc.dram_tensor("xt", [128, NBT * t_steps * 128], BF16,
                          kind="ExternalInput").ap()
    wh_d = nc.dram_tensor("wh", [128, 2048], BF16, kind="ExternalInput").ap()
    wi_d = nc.dram_tensor("wi", [128, 1024], BF16, kind="ExternalInput").ap()
    mlpw_d = nc.dram_tensor("mlpw", [128, NL * 2 * 256], F32R,
                            kind="ExternalInput").ap()
    mlpb_d = nc.dram_tensor("mlpb", [128, NL * 2], F32,
                            kind="ExternalInput").ap()
    wout_d = nc.dram_tensor("wout", [128, 128], F32R, kind="ExternalInput").ap()
    boutb_d = nc.dram_tensor("boutb", [128, 256], F32,
                             kind="ExternalInput").ap()
    pred_d = nc.dram_tensor("pred", [BC, OUT], F32, kind="ExternalOutput").ap()

    with tile.TileContext(nc) as tc:
        with (
            tc.tile_pool(name="singles", bufs=1) as singles,
            tc.tile_pool(name="hpool", bufs=3) as hpool,
            tc.tile_pool(name="work", bufs=3) as work,
            tc.tile_pool(name="psum", bufs=1, space="PSUM") as psum,
        ):
            # ---- weights / persistent state ----
            wh = singles.tile([128, 2048], BF16, tag="wh")
            nc.sync.dma_start(wh[:], wh_d[:])
            wi = singles.tile([128, 1024], BF16, tag="wi")
            nc.sync.dma_start(wi[:], wi_d[:])
            mlpw = singles.tile([128, NL * 2 * 256], F32R, tag="mlpw")
            nc.sync.dma_start(mlpw[:], mlpw_d[:])
            mlpb = singles.tile([128, NL * 2], F32, tag="mlpb")
            nc.sync.dma_start(mlpb[:], mlpb_d[:])
            wout = singles.tile([128, 128], F32R, tag="wout")
            nc.sync.dma_start(wout[:], wout_d[:])
            boutb = singles.tile([128, 256], F32, tag="boutb")
            nc.sync.dma_start(boutb[:], boutb_d[:])

            # x^T, loaded in chunks so step 0 can start early
            xt = singles.tile([128, NBT * t_steps * 128], BF16, tag="xt")
            TBLK = 10 if t_steps % 10 == 0 else t_steps
            for bt in range(NBT):
                for t0 in range(0, t_steps, TBLK):
                    lo = bt * t_steps * 128 + t0 * 128
                    hi = lo + TBLK * 128
                    nc.sync.dma_start(xt[:, lo:hi], xt_d[:, lo:hi])
            xt_r = xt[:].rearrange("p (bt t b) -> p bt t b", bt=NBT, t=t_steps)

            # c' = 2c state (written at t=0, no memset needed)
            cst = [singles.tile([128, 1024], F32, tag="c0", name="c0")]
            # h'_final for the MLP: [128, (ktile 2) * 512]
            hlast = singles.tile([128, 1024], F32R, tag="hlast")

            def step_phase1(t, ch, h_prev):
                """matmuls + transcendentals, row-tile pipelined.
                psum bank m = m-tile m = z-rows m*128:(m+1)*128, so banks are
                (g0 g1 f0 f1 i0 i1 o0 o1) by (gate, h-row-tile)."""
                zp = psum.tile([128, 4096], F32, tag="z", name="zp")
                rhs_x = xt_r[0:81, :, t:t + 1, :]
                # x-matmuls open each bank's accumulation group early
                # bank order matches the ACT read/release ladder of the
                # previous step so each Wi matmul starts as its bank frees
                for m in [2, 0, 1, 4, 6, 3, 5, 7]:
                    nc.tensor.matmul(
                        zp[:, m * 512:(m + 1) * 512],
                        wi[0:81, m * 128:(m + 1) * 128], rhs_x,
                        start=True, stop=(t == 0))
                if t > 0:
                    # k0 reads h rows 0:128 (ready first); within each
                    # k-group do the g banks then row-tile-0 gate banks
                    # (f0 i0 o0) so tanh_r0 unblocks earliest
                    orders = [[0, 1, 2, 4, 6, 3, 5, 7],
                              [2, 0, 1, 4, 6, 3, 5, 7]]
                    for k in range(2):
                        for m in orders[k]:
                            nc.tensor.matmul(
                                zp[:, m * 512:(m + 1) * 512],
                                wh[:, k * 1024 + m * 128:
                                   k * 1024 + (m + 1) * 128],
                                h_prev[:, k * 512:(k + 1) * 512],
                                start=False, stop=(k == 1))

                # tanh split so the u-path (needs tau_f only) unblocks
                # first, then g (needed by tt), then tau_(i,o) per row-tile
                zpv = zp[:].rearrange("p (gt r c) -> p gt r c", gt=4, c=512)
                tau_f = work.tile([128, 1024], BF16, tag="tau_f",
                                  name="tau_f")
                nc.scalar.activation(tau_f[:, 0:512], zpv[:, 1, 0, :],
                                     AF.Tanh, scale=0.5)
                g = work.tile([128, 1024], BF16, tag="g", name="g")
                nc.scalar.activation(g[:], zp[:, 0:1024], AF.Silu)
                taus = []
                for r in range(2):
                    tau = work.tile([128, 1024], BF16, tag=f"tau_io{r}",
                                    name=f"tau_io{r}")
                    nc.scalar.activation(tau[:, 0:512], zpv[:, 2, r, :],
                                         AF.Tanh, scale=0.5)
                    nc.scalar.activation(tau[:, 512:1024], zpv[:, 3, r, :],
                                         AF.Tanh, scale=0.5)
                    taus.append(tau)
                    if r == 0:
                        nc.scalar.activation(tau_f[:, 512:1024],
                                             zpv[:, 1, 1, :],
                                             AF.Tanh, scale=0.5)
                return g, tau_f, taus

            def step_phase2(t, ch, ph1):
                """row-tile pipelined gate arithmetic: u,tt,c on DVE,
                silu(c) on ACT, h' on GPSIMD"""
                g, tau_f, taus = ph1
                c_t = cst[0]
                last = t == t_steps - 1
                if not last:
                    h_new = hpool.tile([128, 1024], BF16, tag="h", name="h")
                sc = work.tile([128, 1024], BF16, tag="sc", name="sc")
                for r in range(2):
                    sl = slice(r * 512, (r + 1) * 512)
                    tau = taus[r]
                    if t > 0:
                        u = work.tile([128, 512], F32, tag=f"u_{r}",
                                      name=f"u_{r}")
                        nc.vector.scalar_tensor_tensor(
                            u[:], tau_f[:, sl], 1.0, c_t[:, sl],
                            ALU.add, ALU.mult)
                        tt = work.tile([128, 512], BF16, tag=f"t_{r}",
                                       name=f"t_{r}")
                        nc.vector.scalar_tensor_tensor(
                            tt[:], tau[:, 0:512], 1.0, g[:, sl],
                            ALU.add, ALU.mult)
                        nc.vector.scalar_tensor_tensor(
                            c_t[:, sl], u[:], 0.5, tt[:], ALU.mult, ALU.add)
                    else:
                        nc.vector.scalar_tensor_tensor(
                            c_t[:, sl], tau[:, 0:512], 1.0, g[:, sl],
                            ALU.add, ALU.mult)
                    nc.scalar.activation(sc[:, sl], c_t[:, sl], AF.Silu,
                                         scale=0.5)
                    if not last:
                        nc.vector.scalar_tensor_tensor(
                            h_new[:, sl], tau[:, 512:1024], 1.0, sc[:, sl],
                            ALU.add, ALU.mult)
                    else:
                        # DVE here: fp32r-rounded output path is verified
                        nc.vector.scalar_tensor_tensor(
                            hlast[:, sl], tau[:, 512:1024], 1.0, sc[:, sl],
                            ALU.add, ALU.mult)
                return None if last else h_new

            hp = None
            for t in range(t_steps):
                ph1 = step_phase1(t, 0, hp)
                hp = step_phase2(t, 0, ph1)

            # ---- MLP head (fp32r, full 512 batch) ----
            cur = hlast
            for layer in range(NL):
                mp = psum.tile([128, 1024], F32, tag="z",
                               name=f"mlp_ps{layer}")
                for m in range(2):
                    for k in range(2):
                        nc.tensor.matmul(
                            mp[:, m * 512:(m + 1) * 512],
                            mlpw[:, (layer * 2 + k) * 256 + m * 128:
                                 (layer * 2 + k) * 256 + (m + 1) * 128
                                 ],
                            cur[:, k * 512:(k + 1) * 512],
                            start=(k == 0), stop=(k == 1))
                nxt = work.tile([128, 1024], F32R, tag="mlp_out")
                for m in range(2):
                    nc.scalar.activation(
                        nxt[:, m * 512:(m + 1) * 512],
                        mp[:, m * 512:(m + 1) * 512], AF.Silu,
                        bias=mlpb[:, layer * 2 + m:layer * 2 + m + 1])
                cur = nxt

            # output layer back to [batch, OUT] layout:
            # lhsT = activations (stationary), rhs = Wout (moving)
            # all 8 matmuls share one PSUM bank -> one accumulation group
            pp = psum.tile([128, 256], F32, tag="z", name="pred_ps")
            for m in range(4):
                for k in range(2):
                    nc.tensor.matmul(
                        pp[:, m * 64:(m + 1) * 64],
                        cur[:, k * 512 + m * 128:k * 512 + (m + 1) * 128
                            ],
                        wout[:, k * 64:(k + 1) * 64],
                        start=(m == 0 and k == 0), stop=(m == 3 and k == 1))
            preds = singles.tile([128, 256], F32, tag="preds")
            nc.vector.tensor_add(preds[:], pp[:], boutb[:])
            nc.sync.dma_start(
                pred_d.rearrange("(m p) f -> p m f", p=128),
                preds[:].rearrange("p (m f) -> p m f", f=OUT))

    nc.compile()
    _CACHE[t_steps] = nc
    return nc


def _prep_inputs(trajectory, actions, Wi, Wh, bh, mlp_W, mlp_b, Wout, bout,
                 t_steps=T):
    """Host-side layout prep. Returns per-core input maps."""
    f32 = np.float32
    trajectory = np.asarray(trajectory, f32)
    actions = np.asarray(actions, f32)
    Wi = np.asarray(Wi, f32)
    Wh = np.asarray(Wh, f32)
    bh = np.asarray(bh, f32)
    mlp_W = np.asarray(mlp_W, f32)
    mlp_b = np.asarray(mlp_b, f32)
    Wout = np.asarray(Wout, f32)
    bout = np.asarray(bout, f32)

    # gate permutation + h'=2h folding
    Wh_p = (0.5 * Wh[:, _PERM]).astype(ml_dtypes.bfloat16)
    Wi_p = Wi[:, _PERM]
    bh_p = bh[_PERM]

    wh_l = Wh_p.reshape(2, 128, 1024).transpose(1, 0, 2).reshape(128, 2048)
    wi_l = np.zeros((128, 1024), ml_dtypes.bfloat16)
    wi_l[0:OBS] = Wi_p[0:OBS].astype(ml_dtypes.bfloat16)
    wi_l[OBS:OBS + ACTD] = Wi_p[OBS:OBS + ACTD].astype(ml_dtypes.bfloat16)
    wi_l[80] = bh_p.astype(ml_dtypes.bfloat16)

    mw = mlp_W.copy()
    mw[0] = mw[0] * 0.5  # h' = 2h fold
    mlpw_l = mw.reshape(NL, 2, 128, 256).transpose(2, 0, 1, 3).reshape(
        128, NL * 2 * 256)
    mlpb_l = mlp_b.reshape(NL, 2, 128).transpose(2, 0, 1).reshape(128, NL * 2)
    wout_l = Wout.reshape(2, 128, 64).transpose(1, 0, 2).reshape(128, 128)
    boutb_l = np.tile(bout, (128, 4))

    in_maps = []
    for c in range(N_CORES):
        tr = trajectory[c * BC:(c + 1) * BC, :t_steps]    # [512, t, 64]
        ac = actions[c * BC:(c + 1) * BC, :t_steps]       # [512, t, 16]
        xt = np.zeros((128, NBT, t_steps, 128), ml_dtypes.bfloat16)
        xt[0:OBS] = tr.reshape(NBT, 128, t_steps, OBS).transpose(
            3, 0, 2, 1).astype(ml_dtypes.bfloat16)
        xt[OBS:OBS + ACTD] = ac.reshape(NBT, 128, t_steps, ACTD).transpose(
            3, 0, 2, 1).astype(ml_dtypes.bfloat16)
        xt[80] = 1.0
        in_maps.append({
            "xt": xt.reshape(128, NBT * t_steps * 128),
            "wh": wh_l, "wi": wi_l, "mlpw": mlpw_l.astype(f32),
            "mlpb": mlpb_l.astype(f32), "wout": wout_l.astype(f32),
            "boutb": boutb_l.astype(f32),
        })
    return in_maps


_RUNNER = {}


def _get_runner(t_steps=T):
    """Build the bass module once and wrap it in a cached, reusable
    shard-mapped PJRT executable (one NEFF compile per process)."""
    if t_steps in _RUNNER:
        return _RUNNER[t_steps]

    import jax
    from jax.sharding import Mesh, PartitionSpec
    from jax.experimental.shard_map import shard_map
    from concourse import bass2jax, mybir as _mb

    nc = _build(t_steps)
    bass2jax.install_neuronx_cc_hook()

    part_name = (nc.partition_id_tensor.name if nc.partition_id_tensor
                 else None)
    in_names, out_names, out_avals = [], [], []
    for alloc in nc.m.functions[0].allocations:
        if not isinstance(alloc, _mb.MemoryLocationSet):
            continue
        name = alloc.memorylocations[0].name
        if alloc.kind == "ExternalInput":
            if name != part_name:
                in_names.append(name)
        elif alloc.kind == "ExternalOutput":
            out_names.append(name)
            out_avals.append(jax.core.ShapedArray(
                tuple(alloc.tensor_shape), _mb.dt.np(alloc.dtype)))
    n_params = len(in_names)
    n_outs = len(out_avals)
    all_names = in_names + out_names
    if part_name is not None:
        all_names = all_names + [part_name]

    def _body(*args):
        operands = list(args)
        if part_name is not None:
            operands.append(bass2jax.partition_id_tensor())
        outs = bass2jax._bass_exec_p.bind(
            *operands,
            out_avals=tuple(out_avals),
            in_names=tuple(all_names),
            out_names=tuple(out_names),
            lowering_input_output_aliases=(),
            sim_require_finite=True,
            sim_require_nnan=True,
            nc=nc,
        )
        return tuple(outs)

    devices = jax.devices()[:N_CORES]
    mesh = Mesh(np.asarray(devices), ("core",))
    donate = tuple(range(n_params, n_params + n_outs))
    sharded = jax.jit(
        shard_map(_body, mesh=mesh,
                  in_specs=(PartitionSpec("core"),) * (n_params + n_outs),
                  out_specs=(PartitionSpec("core"),) * n_outs,
                  check_rep=False),
        donate_argnums=donate, keep_unused=True)

    sharded_nodon = jax.jit(
        shard_map(_body, mesh=mesh,
                  in_specs=(PartitionSpec("core"),) * (n_params + n_outs),
                  out_specs=(PartitionSpec("core"),) * n_outs,
                  check_rep=False),
        keep_unused=True)

    out_shapes = [(a.shape, a.dtype) for a in out_avals]

    def run(in_maps):
        concat_in = [
            np.concatenate([np.asarray(in_maps[c][nm]) for c in
                            range(N_CORES)], axis=0)
            for nm in in_names
        ]
        zeros = [np.zeros((N_CORES * s[0],) + tuple(s[1:]), dt)
                 for s, dt in out_shapes]
        outs = sharded(*concat_in, *zeros)
        return {nm: np.asarray(outs[i]) for i, nm in enumerate(out_names)}

    run.in_names = in_names
    run.mesh = mesh
    run.nodon = sharded_nodon
    run.out_shapes = out_shapes
    _RUNNER[t_steps] = run
    return run


def _stage_inputs(in_maps, t_steps=T):
    """device_put concatenated inputs + zero outs once, for repeat timing."""
    import jax
    from jax.sharding import NamedSharding, PartitionSpec
    run = _get_runner(t_steps)
    sh = NamedSharding(run.mesh, PartitionSpec("core"))
    concat_in = [
        np.concatenate([np.asarray(in_maps[c][nm]) for c in range(N_CORES)],
                       axis=0)
        for nm in run.in_names
    ]
    zeros = [np.zeros((N_CORES * s[0],) + tuple(s[1:]), dt)
             for s, dt in run.out_shapes]
    return [jax.device_put(a, sh) for a in concat_in + zeros], run


def _run_staged(staged):
    arrs, run = staged
    return run.nodon(*arrs)


def kernel(trajectory, actions, Wi, Wh, bh, mlp_W, mlp_b, Wout, bout):
    run = _get_runner(T)
    in_maps = _prep_inputs(trajectory, actions, Wi, Wh, bh, mlp_W, mlp_b,
                           Wout, bout, T)
    pred = run(in_maps)["pred"]          # [8*512, 64] already batch-ordered
    return pred.astype(np.float32)
